# revision 6
# baseline (speedup 1.0000x reference)
"""Lowpass (leaky integrator) scan kernel for Trainium2, 8 NeuronCores.

Recurrence (per feature n, per batch b):
    a_n = exp(-dt / max(tau_n, 1e-8))
    x_t = a_n * x_{t-1} + (1 - a_n) * u_t,   x_{-1} = initial_level_n

Strategy:
  - Data-parallel over batch: 32 batches -> 4 per core, no collectives.
  - On-chip layout: features (N=128) on SBUF partitions, time on the free
    dimension, so the native VectorEngine tensor_tensor_scan instruction
    evaluates the recurrence (state = a*state + v) along time.
  - Input arrives [T, N] (time-major); 128x128 tiles are transposed to
    [N, T] via the TensorEngine (fp32 DMA-transpose unsupported).
  - We scan the rescaled variable z_t = a*z_{t-1} + u_t with
    z_{-1} = x0/(1-a) and x_t = (1-a)*z_t, so the forward transpose feeds
    the scan directly from PSUM with no pre-scaling pass.
  - Back-transpose [N,T] -> [T,N] again on TensorEngine; ScalarEngine
    copies PSUM->SBUF; HWDGE DMA moves 256KB blocks both ways.
"""

import numpy as np
from contextlib import ExitStack

import concourse.bass as bass
import concourse.bacc as bacc
import concourse.mybir as mybir
import concourse.tile as tile
from concourse import masks
from concourse.bass_utils import run_bass_kernel_spmd

DT = 0.001
B, T, N = 32, 4096, 128
NCORES = 8
BC = B // NCORES      # batches per core
TC = 512              # time columns per block (one PSUM bank of fp32)
NSUB = TC // 128      # 128x128 transposes per block
NK = T // TC          # time blocks per batch

_F32 = mybir.dt.float32


def build_nc():
    nc = bacc.Bacc("TRN2", target_bir_lowering=False, debug=False)
    u = nc.declare_dram_parameter("u", [BC, T, N], _F32, isOutput=False)
    tau = nc.declare_dram_parameter("tau", [1, N], _F32, isOutput=False)
    x0 = nc.declare_dram_parameter("x0", [1, N], _F32, isOutput=False)
    y = nc.declare_dram_parameter("y", [BC, T, N], _F32, isOutput=True)

    with tile.TileContext(nc) as tc, ExitStack() as ctx:
        const = ctx.enter_context(tc.tile_pool(name="const", bufs=1))
        in_pool = ctx.enter_context(tc.tile_pool(name="uin", bufs=4))
        z_pool = ctx.enter_context(tc.tile_pool(name="z", bufs=10))
        zs_pool = ctx.enter_context(tc.tile_pool(name="zs", bufs=3))
        out_pool = ctx.enter_context(tc.tile_pool(name="yout", bufs=4))
        pin_pool = ctx.enter_context(tc.tile_pool(name="pin", bufs=3, space="PSUM"))
        pout_pool = ctx.enter_context(tc.tile_pool(name="pout", bufs=3, space="PSUM"))

        ident = const.tile([128, 128], _F32)
        masks.make_identity(nc, ident[:])

        tau_col = const.tile([128, 1], _F32)
        x0_col = const.tile([128, 1], _F32)
        nc.sync.dma_start(tau_col[:], tau[:].rearrange("o n -> n o"))
        nc.sync.dma_start(x0_col[:], x0[:].rearrange("o n -> n o"))

        a_col = const.tile([128, 1], _F32)
        oma_col = const.tile([128, 1], _F32)    # 1 - a
        z0_col = const.tile([128, 1], _F32)     # x0 / (1 - a)
        tmp = const.tile([128, 1], _F32)
        tmp2 = const.tile([128, 1], _F32)

        nc.vector.tensor_scalar_max(tmp[:], tau_col[:], 1e-8)
        nc.vector.reciprocal(tmp[:], tmp[:])
        nc.scalar.activation(
            a_col[:], tmp[:], mybir.ActivationFunctionType.Exp, scale=-DT
        )
        nc.vector.tensor_scalar(
            oma_col[:], a_col[:], -1.0, 1.0,
            op0=mybir.AluOpType.mult, op1=mybir.AluOpType.add,
        )
        nc.vector.reciprocal(tmp2[:], oma_col[:])
        nc.vector.tensor_mul(z0_col[:], x0_col[:], tmp2[:])

        a_bcast = const.tile([128, TC], _F32)
        nc.gpsimd.memset(a_bcast[:], 1.0)
        nc.vector.tensor_scalar_mul(a_bcast[:], a_bcast[:], a_col[:, 0:1])

        prev = [None] * BC
        for k in range(NK):
            for b in range(BC):
                ut = in_pool.tile([128, TC], _F32)
                src = u[b, k * TC:(k + 1) * TC, :].rearrange(
                    "(j p) n -> p j n", p=128
                )
                nc.sync.dma_start(ut[:].rearrange("p (j n) -> p j n", j=NSUB), src)

                pin = pin_pool.tile([128, TC], _F32)
                for j in range(NSUB):
                    nc.tensor.transpose(
                        pin[:, j * 128:(j + 1) * 128],
                        ut[:, j * 128:(j + 1) * 128],
                        ident[:],
                    )

                z = z_pool.tile([128, TC], _F32)
                init = z0_col[:, 0:1] if k == 0 else prev[b][:, TC - 1:TC]
                nc.vector.tensor_tensor_scan(
                    z[:], a_bcast[:], pin[:], init,
                    mybir.AluOpType.mult, mybir.AluOpType.add,
                )
                prev[b] = z

                zs = zs_pool.tile([128, TC], _F32)
                nc.vector.tensor_scalar_mul(zs[:], z[:], oma_col[:, 0:1])

                pout = pout_pool.tile([128, TC], _F32)
                for j in range(NSUB):
                    nc.tensor.transpose(
                        pout[:, j * 128:(j + 1) * 128],
                        zs[:, j * 128:(j + 1) * 128],
                        ident[:],
                    )

                ot = out_pool.tile([128, TC], _F32)
                nc.scalar.copy(ot[:], pout[:])

                dst = y[b, k * TC:(k + 1) * TC, :].rearrange(
                    "(j p) n -> p j n", p=128
                )
                nc.sync.dma_start(dst, ot[:].rearrange("p (j n) -> p j n", j=NSUB))
    nc.compile()
    return nc


_NC = None


def _get_nc():
    global _NC
    if _NC is None:
        _NC = build_nc()
    return _NC


def make_in_maps(inputs, initial_level, tau):
    inputs = np.ascontiguousarray(inputs, dtype=np.float32)
    initial_level = np.ascontiguousarray(initial_level, dtype=np.float32)
    tau = np.ascontiguousarray(tau, dtype=np.float32)
    return [
        {
            "u": inputs[i * BC:(i + 1) * BC],
            "tau": tau,
            "x0": initial_level,
        }
        for i in range(NCORES)
    ]


def kernel(inputs, initial_level, tau):
    nc = _get_nc()
    in_maps = make_in_maps(inputs, initial_level, tau)
    res = run_bass_kernel_spmd(nc, in_maps, list(range(NCORES))).results
    return np.concatenate([res[i]["y"] for i in range(NCORES)], axis=0)


# revision 9
# speedup vs baseline: 1.0678x; 1.0678x over previous
"""Lowpass (leaky integrator) scan kernel for Trainium2, 8 NeuronCores.

Recurrence (per feature n, per batch b):
    a_n = exp(-dt / max(tau_n, 1e-8))
    x_t = a_n * x_{t-1} + (1 - a_n) * u_t,   x_{-1} = initial_level_n

Strategy:
  - Data-parallel over batch: 32 batches -> 4 per core, no collectives.
  - On-chip layout: features (N=128) on SBUF partitions, time on the free
    dimension, so the native VectorEngine tensor_tensor_scan instruction
    evaluates the recurrence (state = a*state + v) along time.
  - Input arrives [T, N] (time-major); 128x128 tiles are transposed to
    [N, T] via the TensorEngine (fp32 DMA-transpose unsupported).
  - We scan the rescaled variable z_t = a*z_{t-1} + u_t with
    z_{-1} = x0/(1-a) and x_t = (1-a)*z_t, so the forward transpose feeds
    the scan directly from PSUM with no pre-scaling pass.
  - Back-transpose [N,T] -> [T,N] again on TensorEngine; ScalarEngine
    copies PSUM->SBUF; HWDGE DMA moves 256KB blocks both ways.
"""

import numpy as np
from contextlib import ExitStack

import concourse.bass as bass
import concourse.bacc as bacc
import concourse.mybir as mybir
import concourse.tile as tile
from concourse import masks
from concourse.bass_utils import run_bass_kernel_spmd

DT = 0.001
B, T, N = 32, 4096, 128
NCORES = 8
BC = B // NCORES      # batches per core
TC = 512              # time columns per block (one PSUM bank of fp32)
NSUB = TC // 128      # 128x128 transposes per block
NK = T // TC          # time blocks per batch

_F32 = mybir.dt.float32


def build_nc():
    nc = bacc.Bacc("TRN2", target_bir_lowering=False, debug=False)
    u = nc.declare_dram_parameter("u", [BC, T, N], _F32, isOutput=False)
    tau = nc.declare_dram_parameter("tau", [1, N], _F32, isOutput=False)
    x0 = nc.declare_dram_parameter("x0", [1, N], _F32, isOutput=False)
    y = nc.declare_dram_parameter("y", [BC, T, N], _F32, isOutput=True)

    with tile.TileContext(nc) as tc, ExitStack() as ctx:
        const = ctx.enter_context(tc.tile_pool(name="const", bufs=1))
        in_pool = ctx.enter_context(tc.tile_pool(name="uin", bufs=4))
        z_pool = ctx.enter_context(tc.tile_pool(name="z", bufs=10))
        zs_pool = ctx.enter_context(tc.tile_pool(name="zs", bufs=6))
        out_pool = ctx.enter_context(tc.tile_pool(name="yout", bufs=4))
        pin_pool = ctx.enter_context(tc.tile_pool(name="pin", bufs=4, space="PSUM"))
        pout_pool = ctx.enter_context(tc.tile_pool(name="pout", bufs=4, space="PSUM"))

        ident = const.tile([128, 128], _F32)
        masks.make_identity(nc, ident[:])

        tau_col = const.tile([128, 1], _F32)
        x0_col = const.tile([128, 1], _F32)
        nc.sync.dma_start(tau_col[:], tau[:].rearrange("o n -> n o"))
        nc.sync.dma_start(x0_col[:], x0[:].rearrange("o n -> n o"))

        a_col = const.tile([128, 1], _F32)
        oma_col = const.tile([128, 1], _F32)    # 1 - a
        z0_col = const.tile([128, 1], _F32)     # x0 / (1 - a)
        tmp = const.tile([128, 1], _F32)
        tmp2 = const.tile([128, 1], _F32)

        nc.vector.tensor_scalar_max(tmp[:], tau_col[:], 1e-8)
        nc.vector.reciprocal(tmp[:], tmp[:])
        nc.scalar.activation(
            a_col[:], tmp[:], mybir.ActivationFunctionType.Exp, scale=-DT
        )
        nc.vector.tensor_scalar(
            oma_col[:], a_col[:], -1.0, 1.0,
            op0=mybir.AluOpType.mult, op1=mybir.AluOpType.add,
        )
        nc.vector.reciprocal(tmp2[:], oma_col[:])
        nc.vector.tensor_mul(z0_col[:], x0_col[:], tmp2[:])

        a_bcast = const.tile([128, TC], _F32)
        nc.gpsimd.memset(a_bcast[:], 1.0)
        nc.vector.tensor_scalar_mul(a_bcast[:], a_bcast[:], a_col[:, 0:1])

        # Software pipeline: the PE back-transposes run PIPE_LAG iterations
        # behind the forward stage so PE never stalls on scan -> scale.
        PIPE_LAG = 2
        prev = [None] * BC
        pending = []  # (zs_tile, b, k) awaiting back-transpose + store

        def emit_back(zs, b, k):
            pout = pout_pool.tile([128, TC], _F32, name="pout")
            for j in range(NSUB):
                nc.tensor.transpose(
                    pout[:, j * 128:(j + 1) * 128],
                    zs[:, j * 128:(j + 1) * 128],
                    ident[:],
                )
            ot = out_pool.tile([128, TC], _F32, name="ot")
            nc.scalar.copy(ot[:], pout[:])
            dst = y[b, k * TC:(k + 1) * TC, :].rearrange("(j p) n -> p j n", p=128)
            nc.sync.dma_start(dst, ot[:].rearrange("p (j n) -> p j n", j=NSUB))

        for k in range(NK):
            for b in range(BC):
                ut = in_pool.tile([128, TC], _F32, name="ut")
                src = u[b, k * TC:(k + 1) * TC, :].rearrange(
                    "(j p) n -> p j n", p=128
                )
                nc.sync.dma_start(ut[:].rearrange("p (j n) -> p j n", j=NSUB), src)

                pin = pin_pool.tile([128, TC], _F32, name="pin")
                for j in range(NSUB):
                    nc.tensor.transpose(
                        pin[:, j * 128:(j + 1) * 128],
                        ut[:, j * 128:(j + 1) * 128],
                        ident[:],
                    )

                z = z_pool.tile([128, TC], _F32, name="z")
                init = z0_col[:, 0:1] if k == 0 else prev[b][:, TC - 1:TC]
                nc.vector.tensor_tensor_scan(
                    z[:], a_bcast[:], pin[:], init,
                    mybir.AluOpType.mult, mybir.AluOpType.add,
                )
                prev[b] = z

                # x = (1-a) * z on the ScalarEngine (per-partition scale)
                zs = zs_pool.tile([128, TC], _F32, name="zs")
                nc.scalar.mul(zs[:], z[:], oma_col[:, 0:1])

                pending.append((zs, b, k))
                if len(pending) > PIPE_LAG:
                    emit_back(*pending.pop(0))

        for args in pending:
            emit_back(*args)
    nc.compile()
    return nc


_NC = None


def _get_nc():
    global _NC
    if _NC is None:
        _NC = build_nc()
    return _NC


def make_in_maps(inputs, initial_level, tau):
    inputs = np.ascontiguousarray(inputs, dtype=np.float32)
    initial_level = np.ascontiguousarray(initial_level, dtype=np.float32)
    tau = np.ascontiguousarray(tau, dtype=np.float32)
    return [
        {
            "u": inputs[i * BC:(i + 1) * BC],
            "tau": tau,
            "x0": initial_level,
        }
        for i in range(NCORES)
    ]


def kernel(inputs, initial_level, tau):
    nc = _get_nc()
    in_maps = make_in_maps(inputs, initial_level, tau)
    res = run_bass_kernel_spmd(nc, in_maps, list(range(NCORES))).results
    return np.concatenate([res[i]["y"] for i in range(NCORES)], axis=0)
